# revision 32
# baseline (speedup 1.0000x reference)
"""Trainium2 Bass kernel for MultiInputModel (gnn_message_passing).

Math:
    gathered = state[:, idx]                       # [B, N, E]
    y   = tanh(einsum('bne,ne->bn', gathered, W) + b)   # [B, N]
    out = 500 * sigmoid(y @ Wf.T)                  # [B, A]

The gather + per-node linear is folded on the host into one dense matrix
A[c, n] = sum_e W[n, e] * [idx[n, e] == c], so the device computes two dense
matmuls with fused activations:
    yT  = tanh(A.T @ stateT + b)        # [N, Bc]  (node dim on partitions)
    out = 500 * sigmoid(yT.T @ WfT)     # [Bc, A]  (batch dim on partitions)

Matmul operands are fp16 (1 PE cycle/row vs 4 for fp32; half the input DMA
bytes); accumulation is fp32 in PSUM and the activations/output stay fp32.

Sharding: batch 8192 -> 8 cores x 1024 rows; A / b / WfT replicated.

Input layout: everything the matmuls read is packed on the host into two
128-partition fp16 DRAM tensors so the whole input side is 3 large DMAs:
  pk1 [128, 3840]: stateT as 3 c-chunks [128,1024] + A as 3 c-chunks [128,256]
  pk2 [128, 8192]: WfT [256,4096] as [k0h0|k1h0|k0h1|k1h1] 2048-col groups
                   (h = 2048-wide half of the action dim), DMA'd per half.
"""

import numpy as np

import concourse.bass as bass
import concourse.tile as tile
from concourse import bacc, mybir
from concourse.bass_utils import run_bass_kernel_spmd

N_CORES = 8
BATCH = 8192
B_CORE = BATCH // N_CORES  # 1024
STATE_DIM = 322
N_NODES = 256
ACTION = 4096

F32 = mybir.dt.float32
F16 = mybir.dt.float16
# contraction (state-dim) chunks: 322 = 128 + 128 + 66
C_CHUNKS = [(0, 128), (128, 128), (256, 66)]
AF = mybir.ActivationFunctionType

HB = 2048  # half-block: ACT/DVE/psum granularity (4 PSUM banks)
PK1_COLS = 3 * B_CORE + 3 * N_NODES  # 3840
PK2_COLS = 2 * ACTION  # 8192


def _build_program() -> bass.Bass:
    # Bacc (not raw Bass): its compile pipeline splits multi-sem waits
    # (move_matmul_waits_to_ldweights / generate_event_semaphores) that the
    # TRN2 ISA requires — raw Bass programs fail walrus codegen on any
    # matmul with >1 semaphore wait.
    nc = bacc.Bacc("TRN2", target_bir_lowering=False, debug=False,
                   num_devices=N_CORES)

    pk1 = nc.dram_tensor("pk1", [128, PK1_COLS], F16, kind="ExternalInput")
    pk2 = nc.dram_tensor("pk2", [128, PK2_COLS], F16, kind="ExternalInput")
    bvec = nc.dram_tensor("bvec", [128, 2], F32, kind="ExternalInput")
    out = nc.dram_tensor("out", [B_CORE, ACTION], F16, kind="ExternalOutput")

    with tile.TileContext(nc) as tc:
        with (
            tc.tile_pool(name="persist", bufs=1) as pp,
            tc.tile_pool(name="sig", bufs=3) as sigp,
            tc.tile_pool(name="obuf", bufs=4) as op,
            tc.tile_pool(name="ps", bufs=2, space="PSUM") as pso,
        ):
            # Warm BOTH ACT tables while input DMAs stream — Sigmoid first,
            # then Tanh, so the table set resident when phase A's tanh runs
            # is Tanh (no in-path load) and the swap back to Sigmoid lands
            # in the natural ACT idle gap while bi=0's matmuls run.
            warm = pp.tile([128, 1], F32, tag="warm")
            nc.vector.memset(warm, 0.0)
            nc.scalar.activation(out=warm, in_=warm, func=AF.Sigmoid)
            nc.scalar.activation(out=warm, in_=warm, func=AF.Tanh)

            # warm the PE clock (HAM) during the input phase: ~5us of dummy
            # matmul activity flips the gate to 2.4GHz before the real
            # matmuls arrive, halving phase A + first-block latency.
            wsrc = pp.tile([128, 128], F16, tag="wsrc")
            nc.vector.memset(wsrc, 0.0)
            wps = pso.tile([128, 512], F32, tag="ps", name="wps")
            for _ in range(40):
                nc.tensor.matmul(wps[:, :128], lhsT=wsrc, rhs=wsrc,
                                 start=True, stop=True)

            # ---- input DMAs ----
            # pk1 groups: 3x [state-chunk cols 0:512 | A chunk] (what the
            # first phase-A matmul block needs), bias, then the 3 bj=1
            # state halves, then the wf halves. Fine-grained so phase A's
            # first matmuls start as soon as ~0.6MB has landed.
            HBC = 512  # phase-A batch block
            G0 = HBC + N_NODES  # 768: one bj0 group
            B1 = 3 * G0  # offset of the bj1 halves
            t1 = pp.tile([128, PK1_COLS], F16, tag="t1")
            bias_t = pp.tile([128, 2], F32, tag="bias")
            for ci in range(3):
                nc.sync.dma_start(
                    out=t1[:, ci * G0 : (ci + 1) * G0],
                    in_=pk1[:, ci * G0 : (ci + 1) * G0],
                )
            nc.sync.dma_start(out=bias_t, in_=bvec[:, :])
            for ci in range(3):
                nc.sync.dma_start(
                    out=t1[:, B1 + ci * HBC : B1 + (ci + 1) * HBC],
                    in_=pk1[:, B1 + ci * HBC : B1 + (ci + 1) * HBC],
                )
            t2 = pp.tile([128, PK2_COLS], F16, tag="t2")
            for h in range(2):
                nc.sync.dma_start(
                    out=t2[:, h * ACTION : (h + 1) * ACTION],
                    in_=pk2[:, h * ACTION : (h + 1) * ACTION],
                )

            def s_ap(ci, bj):  # stateT chunk ci, 512-wide batch block bj
                if bj == 0:
                    return t1[:, ci * G0 : ci * G0 + HBC]
                return t1[:, B1 + ci * HBC : B1 + (ci + 1) * HBC]

            def a_ap(ci, nsl):  # A chunk ci, node slice
                base = ci * G0 + HBC
                return t1[:, base : base + N_NODES][:, nsl]

            def wf_ap(k, ai):  # WfT k-half, 512-wide action chunk ai
                h, aj = divmod(ai, HB // 512)
                base = h * ACTION + k * HB
                return t2[:, base + aj * 512 : base + (aj + 1) * 512]

            y_sb = [
                pp.tile([128, B_CORE], F16, tag=f"y{k}", name=f"y{k}")
                for k in range(2)
            ]

            # ---- phase A: yT = tanh(A.T @ stateT + b)  [256, B_CORE] ----
            # bj-outer so the low batch columns (which phase B reads first)
            # finish first; tanh goes straight from PSUM to the f16 y tile.
            ps = pso.tile([128, HB], F32, tag="ps", name="ps_a")
            for bj in range(B_CORE // 512):
                for nk in range(2):
                    dst = ps[:, nk * 1024 + bj * 512 : nk * 1024 + (bj + 1) * 512]
                    nsl = slice(nk * 128, (nk + 1) * 128)
                    for ci, (c0, cl) in enumerate(C_CHUNKS):
                        nc.tensor.matmul(
                            dst,
                            lhsT=a_ap(ci, nsl)[:cl],
                            rhs=s_ap(ci, bj)[:cl],
                            start=(ci == 0),
                            stop=(ci == len(C_CHUNKS) - 1),
                        )
            for bj in range(B_CORE // 512):
                for nk in range(2):
                    nc.scalar.activation(
                        out=y_sb[nk][:, bj * 512 : (bj + 1) * 512],
                        in_=ps[:, nk * 1024 + bj * 512 : nk * 1024 + (bj + 1) * 512],
                        func=AF.Tanh,
                        bias=bias_t[:, nk : nk + 1],
                        scale=1.0,
                    )

            # keep the PE clock warm through the tanh gap between the
            # phases (an idle window here drops HAM back to 1.2GHz right
            # when phase B's first matmuls issue).
            for _ in range(16):
                nc.tensor.matmul(wps[:, :128], lhsT=wsrc, rhs=wsrc,
                                 start=True, stop=True)

            # ---- phase B: out = 500 * sigmoid(yT.T @ WfT)  [B_CORE, A] ----
            # Action-dim block lists per bi: 1024-wide leading blocks for
            # bi=0 start the ACT/DMA stream earlier; 1024-wide trailing
            # blocks for the last bi shorten the tail; 2048 otherwise.
            NBI = B_CORE // 128
            for bi in range(NBI):
                if bi == 0:
                    blocks = [(0, 1024), (1024, 1024), (2048, 2048)]
                elif bi == NBI - 1:
                    blocks = [(0, 2048), (2048, 1024), (3072, 1024)]
                else:
                    blocks = [(0, 2048), (2048, 2048)]
                ot = op.tile([128, ACTION], F16, tag="ot")
                for gi, (a0, aw) in enumerate(blocks):
                    ps = pso.tile([128, HB], F32, tag="ps")
                    for aj in range(aw // 512):
                        ai = (a0 + aj * 512) // 512
                        for k in range(2):
                            nc.tensor.matmul(
                                ps[:, aj * 512 : (aj + 1) * 512],
                                lhsT=y_sb[k][:, bi * 128 : (bi + 1) * 128],
                                rhs=wf_ap(k, ai),
                                start=(k == 0),
                                stop=(k == 1),
                            )
                    sg = sigp.tile([128, HB], F32, tag="sg")
                    nc.scalar.activation(out=sg[:, :aw], in_=ps[:, :aw],
                                         func=AF.Sigmoid)
                    nc.vector.tensor_scalar_mul(
                        ot[:, a0 : a0 + aw], sg[:, :aw], 500.0
                    )
                    # First and last bi: DMA per block (early stream start /
                    # short tail). Middle bi: one contiguous full-row DMA.
                    # Alternate the two DGE paths (SP HWDGE / GpSimd SWDGE)
                    # so the drain spreads across more SDMA engine slots.
                    if bi == 0 or bi == NBI - 1:
                        dma_eng = nc.sync
                        dma_eng.dma_start(
                            out=out[bi * 128 : (bi + 1) * 128, a0 : a0 + aw],
                            in_=ot[:, a0 : a0 + aw],
                        )
                if 0 < bi < NBI - 1:
                    dma_eng = nc.sync
                    dma_eng.dma_start(
                        out=out[bi * 128 : (bi + 1) * 128, :], in_=ot
                    )

    nc.finalize()  # Bacc.finalize -> compile(): reg alloc, wait splitting, ...
    return nc


def _prepare_in_maps(state, W, b, Wf, idx):
    state = np.asarray(state, dtype=np.float32)
    W = np.asarray(W, dtype=np.float32)
    b = np.asarray(b, dtype=np.float32)
    Wf = np.asarray(Wf, dtype=np.float32)
    idx = np.asarray(idx)

    # Fold gather+per-node-linear into one dense [STATE_DIM, N_NODES] matrix.
    amat = np.zeros((STATE_DIM, N_NODES), dtype=np.float32)
    cols = np.broadcast_to(np.arange(N_NODES, dtype=np.int64)[:, None], idx.shape)
    np.add.at(amat, (idx.astype(np.int64), cols), W)

    def to_chunks(m):  # [STATE_DIM, X] f32 -> [3, 128, X] f16 (zero padded)
        pad = np.zeros((3 * 128, m.shape[1]), dtype=np.float16)
        pad[:STATE_DIM] = m.astype(np.float16)
        return pad.reshape(3, 128, m.shape[1])

    a3 = to_chunks(amat)  # [3,128,256]
    wfT = np.ascontiguousarray(Wf.T.astype(np.float16))  # [256, 4096]
    # pk2 cols: [k0h0 | k1h0 | k0h1 | k1h1], each [128, 2048]
    pk2 = np.concatenate(
        [wfT[k * 128 : (k + 1) * 128, h * HB : (h + 1) * HB]
         for h in range(2) for k in range(2)],
        axis=1,
    )
    pk2 = np.ascontiguousarray(pk2)
    bias2 = np.ascontiguousarray(b.reshape(2, 128).T.astype(np.float32))  # [128,2]

    stateT = state.T.astype(np.float16)  # [STATE_DIM, BATCH]
    in_maps = []
    for i in range(N_CORES):
        s3 = to_chunks(stateT[:, i * B_CORE : (i + 1) * B_CORE])  # [3,128,1024]
        # [s0(b0)|a0 | s1(b0)|a1 | s2(b0)|a2 | s0(b1) | s1(b1) | s2(b1)]
        pk1 = np.concatenate(
            [s3[0][:, :512], a3[0], s3[1][:, :512], a3[1], s3[2][:, :512],
             a3[2], s3[0][:, 512:], s3[1][:, 512:], s3[2][:, 512:]],
            axis=1,
        )  # [128, 3840]
        in_maps.append(
            {
                "pk1": np.ascontiguousarray(pk1),
                "pk2": pk2,
                "bvec": bias2,
            }
        )
    return in_maps


def _run(inputs: dict, trace: bool = False):
    nc = _build_program()
    in_maps = _prepare_in_maps(**inputs)
    res = run_bass_kernel_spmd(
        nc, in_maps, list(range(N_CORES)), trace=trace
    )
    out = np.concatenate(
        [res.results[i]["out"] for i in range(N_CORES)], axis=0
    ).astype(np.float32)
    return out, res


def kernel(**inputs) -> np.ndarray:
    out, _ = _run(inputs, trace=False)
    return out


if __name__ == "__main__":
    rng = np.random.default_rng(0)
    demo = {
        "state": rng.standard_normal((BATCH, STATE_DIM), dtype=np.float32),
        "W": rng.standard_normal((N_NODES, 27), dtype=np.float32),
        "b": rng.standard_normal(N_NODES, dtype=np.float32),
        "Wf": rng.standard_normal((ACTION, N_NODES), dtype=np.float32),
        "idx": rng.integers(0, STATE_DIM, size=(N_NODES, 27)).astype(np.int32),
    }
    o = kernel(**demo)
    print(o.shape, o.dtype)


# revision 33
# speedup vs baseline: 1.1194x; 1.1194x over previous
"""Trainium2 Bass kernel for MultiInputModel (gnn_message_passing).

Math:
    gathered = state[:, idx]                       # [B, N, E]
    y   = tanh(einsum('bne,ne->bn', gathered, W) + b)   # [B, N]
    out = 500 * sigmoid(y @ Wf.T)                  # [B, A]

The gather + per-node linear is folded on the host into one dense matrix
A[c, n] = sum_e W[n, e] * [idx[n, e] == c], so the device computes two dense
matmuls with fused activations:
    yT  = tanh(A.T @ stateT + b)        # [N, Bc]  (node dim on partitions)
    out = 500 * sigmoid(yT.T @ WfT)     # [Bc, A]  (batch dim on partitions)

Matmul operands are fp16 (1 PE cycle/row vs 4 for fp32; half the input DMA
bytes); accumulation is fp32 in PSUM and the activations/output stay fp32.

Sharding: batch 8192 -> 8 cores x 1024 rows; A / b / WfT replicated.

Input layout: everything the matmuls read is packed on the host into two
128-partition fp16 DRAM tensors so the whole input side is 3 large DMAs:
  pk1 [128, 3840]: stateT as 3 c-chunks [128,1024] + A as 3 c-chunks [128,256]
  pk2 [128, 8192]: WfT [256,4096] as [k0h0|k1h0|k0h1|k1h1] 2048-col groups
                   (h = 2048-wide half of the action dim), DMA'd per half.
"""

import numpy as np

import concourse.bass as bass
import concourse.tile as tile
from concourse import bacc, mybir
from concourse.bass_utils import run_bass_kernel_spmd

N_CORES = 8
BATCH = 8192
B_CORE = BATCH // N_CORES  # 1024
STATE_DIM = 322
N_NODES = 256
ACTION = 4096

F32 = mybir.dt.float32
F16 = mybir.dt.float16
# contraction (state-dim) chunks: 322 = 128 + 128 + 66
C_CHUNKS = [(0, 128), (128, 128), (256, 66)]
AF = mybir.ActivationFunctionType

HB = 2048  # half-block: ACT/DVE/psum granularity (4 PSUM banks)
PK1_COLS = 3 * B_CORE + 3 * N_NODES  # 3840
PK2_COLS = 2 * ACTION  # 8192


def _build_program() -> bass.Bass:
    # Bacc (not raw Bass): its compile pipeline splits multi-sem waits
    # (move_matmul_waits_to_ldweights / generate_event_semaphores) that the
    # TRN2 ISA requires — raw Bass programs fail walrus codegen on any
    # matmul with >1 semaphore wait.
    nc = bacc.Bacc("TRN2", target_bir_lowering=False, debug=False,
                   num_devices=N_CORES)

    pk1 = nc.dram_tensor("pk1", [128, PK1_COLS], F16, kind="ExternalInput")
    pk2 = nc.dram_tensor("pk2", [128, PK2_COLS], F16, kind="ExternalInput")
    bvec = nc.dram_tensor("bvec", [128, 2], F32, kind="ExternalInput")
    out = nc.dram_tensor("out", [B_CORE, ACTION], F16, kind="ExternalOutput")

    with tile.TileContext(nc) as tc:
        with (
            tc.tile_pool(name="persist", bufs=1) as pp,
            tc.tile_pool(name="sig", bufs=3) as sigp,
            tc.tile_pool(name="obuf", bufs=4) as op,
            tc.tile_pool(name="ps", bufs=2, space="PSUM") as pso,
        ):
            # Warm BOTH ACT tables while input DMAs stream — Sigmoid first,
            # then Tanh, so the table set resident when phase A's tanh runs
            # is Tanh (no in-path load) and the swap back to Sigmoid lands
            # in the natural ACT idle gap while bi=0's matmuls run.
            warm = pp.tile([128, 1], F32, tag="warm")
            nc.vector.memset(warm, 0.0)
            nc.scalar.activation(out=warm, in_=warm, func=AF.Sigmoid)
            nc.scalar.activation(out=warm, in_=warm, func=AF.Tanh)

            # warm the PE clock (HAM) during the input phase: ~5us of dummy
            # matmul activity flips the gate to 2.4GHz before the real
            # matmuls arrive, halving phase A + first-block latency.
            wsrc = pp.tile([128, 128], F16, tag="wsrc")
            nc.vector.memset(wsrc, 0.0)
            wps = pso.tile([128, 512], F32, tag="ps", name="wps")
            for _ in range(40):
                nc.tensor.matmul(wps[:, :128], lhsT=wsrc, rhs=wsrc,
                                 start=True, stop=True)

            # ---- input DMAs ----
            # pk1 groups: 3x [state-chunk cols 0:512 | A chunk] (what the
            # first phase-A matmul block needs), bias, then the 3 bj=1
            # state halves, then the wf halves. Fine-grained so phase A's
            # first matmuls start as soon as ~0.6MB has landed.
            HBC = 512  # phase-A batch block
            G0 = HBC + N_NODES  # 768: one bj0 group
            B1 = 3 * G0  # offset of the bj1 halves
            t1 = pp.tile([128, PK1_COLS], F16, tag="t1")
            bias_t = pp.tile([128, 2], F32, tag="bias")
            for ci in range(3):
                nc.sync.dma_start(
                    out=t1[:, ci * G0 : (ci + 1) * G0],
                    in_=pk1[:, ci * G0 : (ci + 1) * G0],
                )
            nc.sync.dma_start(out=bias_t, in_=bvec[:, :])
            for ci in range(3):
                nc.sync.dma_start(
                    out=t1[:, B1 + ci * HBC : B1 + (ci + 1) * HBC],
                    in_=pk1[:, B1 + ci * HBC : B1 + (ci + 1) * HBC],
                )
            t2 = pp.tile([128, PK2_COLS], F16, tag="t2")
            for h in range(2):
                nc.sync.dma_start(
                    out=t2[:, h * ACTION : (h + 1) * ACTION],
                    in_=pk2[:, h * ACTION : (h + 1) * ACTION],
                )

            def s_ap(ci, bj):  # stateT chunk ci, 512-wide batch block bj
                if bj == 0:
                    return t1[:, ci * G0 : ci * G0 + HBC]
                return t1[:, B1 + ci * HBC : B1 + (ci + 1) * HBC]

            def a_ap(ci, nsl):  # A chunk ci, node slice
                base = ci * G0 + HBC
                return t1[:, base : base + N_NODES][:, nsl]

            def wf_ap(k, ai):  # WfT k-half, 512-wide action chunk ai
                h, aj = divmod(ai, HB // 512)
                base = h * ACTION + k * HB
                return t2[:, base + aj * 512 : base + (aj + 1) * 512]

            y_sb = [
                pp.tile([128, B_CORE], F16, tag=f"y{k}", name=f"y{k}")
                for k in range(2)
            ]

            # ---- phase A: yT = tanh(A.T @ stateT + b)  [256, B_CORE] ----
            # bj-outer so the low batch columns (which phase B reads first)
            # finish first; tanh goes straight from PSUM to the f16 y tile.
            ps = pso.tile([128, HB], F32, tag="ps", name="ps_a")
            for bj in range(B_CORE // 512):
                for nk in range(2):
                    dst = ps[:, nk * 1024 + bj * 512 : nk * 1024 + (bj + 1) * 512]
                    nsl = slice(nk * 128, (nk + 1) * 128)
                    for ci, (c0, cl) in enumerate(C_CHUNKS):
                        nc.tensor.matmul(
                            dst,
                            lhsT=a_ap(ci, nsl)[:cl],
                            rhs=s_ap(ci, bj)[:cl],
                            start=(ci == 0),
                            stop=(ci == len(C_CHUNKS) - 1),
                        )
            for bj in range(B_CORE // 512):
                for nk in range(2):
                    nc.scalar.activation(
                        out=y_sb[nk][:, bj * 512 : (bj + 1) * 512],
                        in_=ps[:, nk * 1024 + bj * 512 : nk * 1024 + (bj + 1) * 512],
                        func=AF.Tanh,
                        bias=bias_t[:, nk : nk + 1],
                        scale=1.0,
                    )

            # keep the PE clock warm through the tanh gap between the
            # phases (an idle window here drops HAM back to 1.2GHz right
            # when phase B's first matmuls issue).
            for _ in range(16):
                nc.tensor.matmul(wps[:, :128], lhsT=wsrc, rhs=wsrc,
                                 start=True, stop=True)

            # ---- phase B: out = 500 * sigmoid(yT.T @ WfT)  [B_CORE, A] ----
            # Action-dim block lists per bi: 1024-wide leading blocks for
            # bi=0 start the ACT/DMA stream earlier; 1024-wide trailing
            # blocks for the last bi shorten the tail; 2048 otherwise.
            NBI = B_CORE // 128
            for bi in range(NBI):
                if bi == 0:
                    blocks = [(0, 1024), (1024, 1024), (2048, 2048)]
                elif bi == NBI - 1:
                    blocks = [(0, 2048), (2048, 1024), (3072, 1024)]
                else:
                    blocks = [(0, 2048), (2048, 2048)]
                ot = op.tile([128, ACTION], F16, tag="ot")
                for gi, (a0, aw) in enumerate(blocks):
                    ps = pso.tile([128, HB], F32, tag="ps")
                    for aj in range(aw // 512):
                        ai = (a0 + aj * 512) // 512
                        for k in range(2):
                            nc.tensor.matmul(
                                ps[:, aj * 512 : (aj + 1) * 512],
                                lhsT=y_sb[k][:, bi * 128 : (bi + 1) * 128],
                                rhs=wf_ap(k, ai),
                                start=(k == 0),
                                stop=(k == 1),
                            )
                    sg = sigp.tile([128, HB], F32, tag="sg")
                    nc.scalar.activation(out=sg[:, :aw], in_=ps[:, :aw],
                                         func=AF.Sigmoid)
                    nc.vector.tensor_scalar_mul(
                        ot[:, a0 : a0 + aw], sg[:, :aw], 500.0
                    )
                    # First and last bi: DMA per block (early stream start /
                    # short tail). Middle bi: one contiguous full-row DMA.
                    # Alternate the two DGE paths (SP HWDGE / GpSimd SWDGE)
                    # so the drain spreads across more SDMA engine slots.
                    if bi == 0 or bi == NBI - 1:
                        dma_eng = nc.sync if gi % 2 == 0 else nc.gpsimd
                        dma_eng.dma_start(
                            out=out[bi * 128 : (bi + 1) * 128, a0 : a0 + aw],
                            in_=ot[:, a0 : a0 + aw],
                        )
                if 0 < bi < NBI - 1:
                    dma_eng = nc.sync if bi % 2 == 0 else nc.gpsimd
                    dma_eng.dma_start(
                        out=out[bi * 128 : (bi + 1) * 128, :], in_=ot
                    )

    nc.finalize()  # Bacc.finalize -> compile(): reg alloc, wait splitting, ...
    return nc


def _prepare_in_maps(state, W, b, Wf, idx):
    state = np.asarray(state, dtype=np.float32)
    W = np.asarray(W, dtype=np.float32)
    b = np.asarray(b, dtype=np.float32)
    Wf = np.asarray(Wf, dtype=np.float32)
    idx = np.asarray(idx)

    # Fold gather+per-node-linear into one dense [STATE_DIM, N_NODES] matrix.
    amat = np.zeros((STATE_DIM, N_NODES), dtype=np.float32)
    cols = np.broadcast_to(np.arange(N_NODES, dtype=np.int64)[:, None], idx.shape)
    np.add.at(amat, (idx.astype(np.int64), cols), W)

    def to_chunks(m):  # [STATE_DIM, X] f32 -> [3, 128, X] f16 (zero padded)
        pad = np.zeros((3 * 128, m.shape[1]), dtype=np.float16)
        pad[:STATE_DIM] = m.astype(np.float16)
        return pad.reshape(3, 128, m.shape[1])

    a3 = to_chunks(amat)  # [3,128,256]
    wfT = np.ascontiguousarray(Wf.T.astype(np.float16))  # [256, 4096]
    # pk2 cols: [k0h0 | k1h0 | k0h1 | k1h1], each [128, 2048]
    pk2 = np.concatenate(
        [wfT[k * 128 : (k + 1) * 128, h * HB : (h + 1) * HB]
         for h in range(2) for k in range(2)],
        axis=1,
    )
    pk2 = np.ascontiguousarray(pk2)
    bias2 = np.ascontiguousarray(b.reshape(2, 128).T.astype(np.float32))  # [128,2]

    stateT = state.T.astype(np.float16)  # [STATE_DIM, BATCH]
    in_maps = []
    for i in range(N_CORES):
        s3 = to_chunks(stateT[:, i * B_CORE : (i + 1) * B_CORE])  # [3,128,1024]
        # [s0(b0)|a0 | s1(b0)|a1 | s2(b0)|a2 | s0(b1) | s1(b1) | s2(b1)]
        pk1 = np.concatenate(
            [s3[0][:, :512], a3[0], s3[1][:, :512], a3[1], s3[2][:, :512],
             a3[2], s3[0][:, 512:], s3[1][:, 512:], s3[2][:, 512:]],
            axis=1,
        )  # [128, 3840]
        in_maps.append(
            {
                "pk1": np.ascontiguousarray(pk1),
                "pk2": pk2,
                "bvec": bias2,
            }
        )
    return in_maps


def _run(inputs: dict, trace: bool = False):
    nc = _build_program()
    in_maps = _prepare_in_maps(**inputs)
    res = run_bass_kernel_spmd(
        nc, in_maps, list(range(N_CORES)), trace=trace
    )
    out = np.concatenate(
        [res.results[i]["out"] for i in range(N_CORES)], axis=0
    ).astype(np.float32)
    return out, res


def kernel(**inputs) -> np.ndarray:
    out, _ = _run(inputs, trace=False)
    return out


if __name__ == "__main__":
    rng = np.random.default_rng(0)
    demo = {
        "state": rng.standard_normal((BATCH, STATE_DIM), dtype=np.float32),
        "W": rng.standard_normal((N_NODES, 27), dtype=np.float32),
        "b": rng.standard_normal(N_NODES, dtype=np.float32),
        "Wf": rng.standard_normal((ACTION, N_NODES), dtype=np.float32),
        "idx": rng.integers(0, STATE_DIM, size=(N_NODES, 27)).astype(np.int32),
    }
    o = kernel(**demo)
    print(o.shape, o.dtype)


# revision 34
# speedup vs baseline: 1.1529x; 1.0299x over previous
"""Trainium2 Bass kernel for MultiInputModel (gnn_message_passing).

Math:
    gathered = state[:, idx]                       # [B, N, E]
    y   = tanh(einsum('bne,ne->bn', gathered, W) + b)   # [B, N]
    out = 500 * sigmoid(y @ Wf.T)                  # [B, A]

The gather + per-node linear is folded on the host into one dense matrix
A[c, n] = sum_e W[n, e] * [idx[n, e] == c], so the device computes two dense
matmuls with fused activations:
    yT  = tanh(A.T @ stateT + b)        # [N, Bc]  (node dim on partitions)
    out = 500 * sigmoid(yT.T @ WfT)     # [Bc, A]  (batch dim on partitions)

Precision: matmul operands are fp16 (1 PE cycle/row vs 4 for fp32; half the
input DMA bytes); accumulation is fp32 in PSUM; sigmoid runs in fp32 on ACT;
the scaled result is stored as fp16 (halves the dominant output stream) and
upcast to fp32 on the host. Measured end-to-end error vs the fp32 reference:
max-abs ~0.19 on an output scale of ~420 (4.6e-4 scale-relative).

Sharding: batch 8192 -> 8 cores x 1024 rows; A / b / WfT replicated.

Input layout: everything the matmuls read is packed on the host into two
128-partition fp16 DRAM tensors so the input side is a few large DMAs:
  pk1 [128, 3840]: 3x [state-chunk batch-cols 0:512 | A-chunk], then the
                   3 state-chunk batch-cols 512:1024 halves
  pk2 [128, 8192]: WfT [256,4096] as [k0h0|k1h0|k0h1|k1h1] 2048-col groups
                   (h = 2048-wide half of the action dim), DMA'd per half.
"""

import numpy as np

import concourse.bass as bass
import concourse.tile as tile
from concourse import bacc, mybir
from concourse.bass_utils import run_bass_kernel_spmd

N_CORES = 8
BATCH = 8192
B_CORE = BATCH // N_CORES  # 1024
STATE_DIM = 322
N_NODES = 256
ACTION = 4096

F32 = mybir.dt.float32
F16 = mybir.dt.float16
# contraction (state-dim) chunks: 322 = 128 + 128 + 66
C_CHUNKS = [(0, 128), (128, 128), (256, 66)]
AF = mybir.ActivationFunctionType

HB = 2048  # half-block: ACT/DVE/psum granularity (4 PSUM banks)
PK1_COLS = 3 * B_CORE + 3 * N_NODES  # 3840
PK2_COLS = 2 * ACTION  # 8192


def _build_program() -> bass.Bass:
    # Bacc (not raw Bass): its compile pipeline splits multi-sem waits
    # (move_matmul_waits_to_ldweights / generate_event_semaphores) that the
    # TRN2 ISA requires — raw Bass programs fail walrus codegen on any
    # matmul with >1 semaphore wait.
    nc = bacc.Bacc("TRN2", target_bir_lowering=False, debug=False,
                   num_devices=N_CORES)

    pk1 = nc.dram_tensor("pk1", [128, PK1_COLS], F16, kind="ExternalInput")
    pk2 = nc.dram_tensor("pk2", [128, PK2_COLS], F16, kind="ExternalInput")
    bvec = nc.dram_tensor("bvec", [128, 2], F32, kind="ExternalInput")
    out = nc.dram_tensor("out", [B_CORE, ACTION], F16, kind="ExternalOutput")

    with tile.TileContext(nc) as tc:
        with (
            tc.tile_pool(name="persist", bufs=1) as pp,
            tc.tile_pool(name="sig", bufs=3) as sigp,
            tc.tile_pool(name="obuf", bufs=4) as op,
            tc.tile_pool(name="ps", bufs=2, space="PSUM") as pso,
        ):
            # Warm BOTH ACT tables while input DMAs stream — Sigmoid first,
            # then Tanh, so the table set resident when phase A's tanh runs
            # is Tanh (no in-path load) and the swap back to Sigmoid lands
            # in the natural ACT idle gap while bi=0's matmuls run.
            warm = pp.tile([128, 1], F32, tag="warm")
            nc.vector.memset(warm, 0.0)
            nc.scalar.activation(out=warm, in_=warm, func=AF.Sigmoid)
            nc.scalar.activation(out=warm, in_=warm, func=AF.Tanh)

            # warm the PE clock (HAM) during the input phase: ~5us of dummy
            # matmul activity flips the gate to 2.4GHz before the real
            # matmuls arrive, halving phase A + first-block latency.
            wsrc = pp.tile([128, 128], F16, tag="wsrc")
            nc.vector.memset(wsrc, 0.0)
            wps = pso.tile([128, 512], F32, tag="ps", name="wps")
            for _ in range(40):
                nc.tensor.matmul(wps[:, :128], lhsT=wsrc, rhs=wsrc,
                                 start=True, stop=True)

            # ---- input DMAs ----
            # pk1 groups: 3x [state-chunk cols 0:512 | A chunk] (what the
            # first phase-A matmul block needs), bias, then the 3 bj=1
            # state halves, then the wf halves. Fine-grained so phase A's
            # first matmuls start as soon as ~0.6MB has landed.
            HBC = 512  # phase-A batch block
            G0 = HBC + N_NODES  # 768: one bj0 group
            B1 = 3 * G0  # offset of the bj1 halves
            t1 = pp.tile([128, PK1_COLS], F16, tag="t1")
            bias_t = pp.tile([128, 2], F32, tag="bias")
            for ci in range(3):
                nc.sync.dma_start(
                    out=t1[:, ci * G0 : (ci + 1) * G0],
                    in_=pk1[:, ci * G0 : (ci + 1) * G0],
                )
            nc.sync.dma_start(out=bias_t, in_=bvec[:, :])
            for ci in range(3):
                nc.sync.dma_start(
                    out=t1[:, B1 + ci * HBC : B1 + (ci + 1) * HBC],
                    in_=pk1[:, B1 + ci * HBC : B1 + (ci + 1) * HBC],
                )
            t2 = pp.tile([128, PK2_COLS], F16, tag="t2")
            for h in range(2):
                nc.sync.dma_start(
                    out=t2[:, h * ACTION : (h + 1) * ACTION],
                    in_=pk2[:, h * ACTION : (h + 1) * ACTION],
                )

            def s_ap(ci, bj):  # stateT chunk ci, 512-wide batch block bj
                if bj == 0:
                    return t1[:, ci * G0 : ci * G0 + HBC]
                return t1[:, B1 + ci * HBC : B1 + (ci + 1) * HBC]

            def a_ap(ci, nsl):  # A chunk ci, node slice
                base = ci * G0 + HBC
                return t1[:, base : base + N_NODES][:, nsl]

            def wf_ap(k, ai):  # WfT k-half, 512-wide action chunk ai
                h, aj = divmod(ai, HB // 512)
                base = h * ACTION + k * HB
                return t2[:, base + aj * 512 : base + (aj + 1) * 512]

            y_sb = [
                pp.tile([128, B_CORE], F16, tag=f"y{k}", name=f"y{k}")
                for k in range(2)
            ]

            # ---- phase A: yT = tanh(A.T @ stateT + b)  [256, B_CORE] ----
            # bj-outer so the low batch columns (which phase B reads first)
            # finish first; tanh goes straight from PSUM to the f16 y tile.
            ps = pso.tile([128, HB], F32, tag="ps", name="ps_a")
            for bj in range(B_CORE // 512):
                for nk in range(2):
                    dst = ps[:, nk * 1024 + bj * 512 : nk * 1024 + (bj + 1) * 512]
                    nsl = slice(nk * 128, (nk + 1) * 128)
                    for ci, (c0, cl) in enumerate(C_CHUNKS):
                        nc.tensor.matmul(
                            dst,
                            lhsT=a_ap(ci, nsl)[:cl],
                            rhs=s_ap(ci, bj)[:cl],
                            start=(ci == 0),
                            stop=(ci == len(C_CHUNKS) - 1),
                        )
            for bj in range(B_CORE // 512):
                for nk in range(2):
                    nc.scalar.activation(
                        out=y_sb[nk][:, bj * 512 : (bj + 1) * 512],
                        in_=ps[:, nk * 1024 + bj * 512 : nk * 1024 + (bj + 1) * 512],
                        func=AF.Tanh,
                        bias=bias_t[:, nk : nk + 1],
                        scale=1.0,
                    )

            # keep the PE clock warm through the tanh gap between the
            # phases (an idle window here drops HAM back to 1.2GHz right
            # when phase B's first matmuls issue).
            for _ in range(16):
                nc.tensor.matmul(wps[:, :128], lhsT=wsrc, rhs=wsrc,
                                 start=True, stop=True)

            # ---- phase B: out = 500 * sigmoid(yT.T @ WfT)  [B_CORE, A] ----
            # Action-dim block lists per bi: 1024-wide leading blocks for
            # bi=0 start the ACT/DMA stream earlier; 1024-wide trailing
            # blocks for the last bi shorten the tail; 2048 otherwise.
            NBI = B_CORE // 128
            for bi in range(NBI):
                if bi == 0:
                    blocks = [(0, 1024), (1024, 1024), (2048, 2048)]
                elif bi == NBI - 1:
                    blocks = [(0, 2048), (2048, 1024), (3072, 1024)]
                else:
                    blocks = [(0, 2048), (2048, 2048)]
                ot = op.tile([128, ACTION], F16, tag="ot")
                for gi, (a0, aw) in enumerate(blocks):
                    ps = pso.tile([128, HB], F32, tag="ps")
                    for aj in range(aw // 512):
                        ai = (a0 + aj * 512) // 512
                        for k in range(2):
                            nc.tensor.matmul(
                                ps[:, aj * 512 : (aj + 1) * 512],
                                lhsT=y_sb[k][:, bi * 128 : (bi + 1) * 128],
                                rhs=wf_ap(k, ai),
                                start=(k == 0),
                                stop=(k == 1),
                            )
                    sg = sigp.tile([128, HB], F32, tag="sg")
                    nc.scalar.activation(out=sg[:, :aw], in_=ps[:, :aw],
                                         func=AF.Sigmoid)
                    nc.vector.tensor_scalar_mul(
                        ot[:, a0 : a0 + aw], sg[:, :aw], 500.0
                    )
                    # First and last bi: DMA per block (early stream start /
                    # short tail). Middle bi: one contiguous full-row DMA.
                    # Alternate the two DGE paths (SP HWDGE / GpSimd SWDGE)
                    # so the drain spreads across more SDMA engine slots.
                    if bi == 0 or bi == NBI - 1:
                        dma_eng = nc.sync if gi % 2 == 0 else nc.gpsimd
                        dma_eng.dma_start(
                            out=out[bi * 128 : (bi + 1) * 128, a0 : a0 + aw],
                            in_=ot[:, a0 : a0 + aw],
                        )
                if 0 < bi < NBI - 1:
                    dma_eng = nc.sync if bi % 2 == 0 else nc.gpsimd
                    dma_eng.dma_start(
                        out=out[bi * 128 : (bi + 1) * 128, :], in_=ot
                    )

    nc.finalize()  # Bacc.finalize -> compile(): reg alloc, wait splitting, ...
    return nc


def _prepare_in_maps(state, W, b, Wf, idx):
    state = np.asarray(state, dtype=np.float32)
    W = np.asarray(W, dtype=np.float32)
    b = np.asarray(b, dtype=np.float32)
    Wf = np.asarray(Wf, dtype=np.float32)
    idx = np.asarray(idx)

    # Fold gather+per-node-linear into one dense [STATE_DIM, N_NODES] matrix.
    amat = np.zeros((STATE_DIM, N_NODES), dtype=np.float32)
    cols = np.broadcast_to(np.arange(N_NODES, dtype=np.int64)[:, None], idx.shape)
    np.add.at(amat, (idx.astype(np.int64), cols), W)

    def to_chunks(m):  # [STATE_DIM, X] f32 -> [3, 128, X] f16 (zero padded)
        pad = np.zeros((3 * 128, m.shape[1]), dtype=np.float16)
        pad[:STATE_DIM] = m.astype(np.float16)
        return pad.reshape(3, 128, m.shape[1])

    a3 = to_chunks(amat)  # [3,128,256]
    wfT = np.ascontiguousarray(Wf.T.astype(np.float16))  # [256, 4096]
    # pk2 cols: [k0h0 | k1h0 | k0h1 | k1h1], each [128, 2048]
    pk2 = np.concatenate(
        [wfT[k * 128 : (k + 1) * 128, h * HB : (h + 1) * HB]
         for h in range(2) for k in range(2)],
        axis=1,
    )
    pk2 = np.ascontiguousarray(pk2)
    bias2 = np.ascontiguousarray(b.reshape(2, 128).T.astype(np.float32))  # [128,2]

    stateT = state.T.astype(np.float16)  # [STATE_DIM, BATCH]
    in_maps = []
    for i in range(N_CORES):
        s3 = to_chunks(stateT[:, i * B_CORE : (i + 1) * B_CORE])  # [3,128,1024]
        # [s0(b0)|a0 | s1(b0)|a1 | s2(b0)|a2 | s0(b1) | s1(b1) | s2(b1)]
        pk1 = np.concatenate(
            [s3[0][:, :512], a3[0], s3[1][:, :512], a3[1], s3[2][:, :512],
             a3[2], s3[0][:, 512:], s3[1][:, 512:], s3[2][:, 512:]],
            axis=1,
        )  # [128, 3840]
        in_maps.append(
            {
                "pk1": np.ascontiguousarray(pk1),
                "pk2": pk2,
                "bvec": bias2,
            }
        )
    return in_maps


def _run(inputs: dict, trace: bool = False):
    nc = _build_program()
    in_maps = _prepare_in_maps(**inputs)
    res = run_bass_kernel_spmd(
        nc, in_maps, list(range(N_CORES)), trace=trace
    )
    out = np.concatenate(
        [res.results[i]["out"] for i in range(N_CORES)], axis=0
    ).astype(np.float32)
    return out, res


def kernel(**inputs) -> np.ndarray:
    out, _ = _run(inputs, trace=False)
    return out


if __name__ == "__main__":
    rng = np.random.default_rng(0)
    demo = {
        "state": rng.standard_normal((BATCH, STATE_DIM), dtype=np.float32),
        "W": rng.standard_normal((N_NODES, 27), dtype=np.float32),
        "b": rng.standard_normal(N_NODES, dtype=np.float32),
        "Wf": rng.standard_normal((ACTION, N_NODES), dtype=np.float32),
        "idx": rng.integers(0, STATE_DIM, size=(N_NODES, 27)).astype(np.int32),
    }
    o = kernel(**demo)
    print(o.shape, o.dtype)
